# revision 1
# baseline (speedup 1.0000x reference)
"""RNN-T Joint network kernel for Trainium2 (Bass/Tile), 8-core data-parallel.

Math (per batch b):
  hf = f[b] @ W1[:1024]            # (T=256, J=640)
  hg = g[b] @ W1[1024:]            # (U=65,  J=640)
  h[t,u,:]   = relu(hf[t] + hg[u] + b1)
  out[t,u,:] = h[t,u,:] @ W2 + b2  # (256, 65, 1024)

Device layout (per core, u-major):
  - hfT[j, t] and hgT'[j, u] = hgT + b1 kept resident in SBUF (j on partitions).
  - For each u: H_u[j, t] = relu(hfT[j, t] + hgT'[j, u]) built by ScalarE
    (bias = per-partition column hgT'[:, u]), cast to bf16.
  - PE: out_tile[t128, v512] += H_u[jc][:, t128].T @ W2bf[jc][:, v512], 5 j-chunks
    accumulated in PSUM (fp32).
  - VectorE drains PSUM + adds broadcast b2, DMA straight to HBM.
"""

import numpy as np

T, U = 256, 65
EH, PH, J, V = 1024, 320, 640, 1024
JC = J // 128           # 5 j-chunks
HC = EH // 128          # 8 h-chunks
N_CORES = 8

_CACHE = {}


def _build_nc():
    import concourse.bass as bass
    import concourse.bacc as bacc
    import concourse.mybir as mybir
    from concourse import tile, masks

    f32 = mybir.dt.float32
    bf16 = mybir.dt.bfloat16
    Relu = mybir.ActivationFunctionType.Relu
    add = mybir.AluOpType.add

    nc = bacc.Bacc(None, target_bir_lowering=False)

    f_d = nc.declare_dram_parameter("f", [T, EH], f32, isOutput=False)
    g_d = nc.declare_dram_parameter("g", [U, PH], f32, isOutput=False)
    W1_d = nc.declare_dram_parameter("W1", [EH + PH, J], f32, isOutput=False)
    b1_d = nc.declare_dram_parameter("b1", [J], f32, isOutput=False)
    W2_d = nc.declare_dram_parameter("W2", [J, V], f32, isOutput=False)
    b2_d = nc.declare_dram_parameter("b2", [V], f32, isOutput=False)
    out_d = nc.declare_dram_parameter("out", [T, U, V], f32, isOutput=True)

    # W1g partition chunks (PH = 320 = 128 + 128 + 64)
    g_chunks = [(0, 128), (128, 128), (256, 64)]

    with tile.TileContext(nc) as tc:
        with tc.tile_pool(name="const", bufs=1) as cpool:
            identity = cpool.tile([128, 128], f32)
            masks.make_identity(nc, identity[:])

            # ---------------- prologue: weights + first layer ----------------
            W2b = []      # bf16 [128, V] x JC
            fTb = []      # bf16 [128, T] x HC   (f^T)
            hfTs = []     # f32  [128, T] x JC   (hf^T)
            hgTs = []     # f32  [128, U] x JC   (hg^T + b1)

            with (
                tc.tile_pool(name="scratch", bufs=2) as spool,
                tc.tile_pool(name="ppsumA", bufs=2, space=bass.MemorySpace.PSUM) as ppA,
                tc.tile_pool(name="ppsumB", bufs=1, space=bass.MemorySpace.PSUM) as ppB,
                tc.tile_pool(name="ppsumC", bufs=2, space=bass.MemorySpace.PSUM) as ppC,
            ):
                # f first: longest dependency chain (load -> transpose -> hfT)
                fraw = []
                for tt in range(2):
                    t = spool.tile([128, EH], f32, tag=f"fraw{tt}")
                    nc.sync.dma_start(out=t[:], in_=f_d[tt * 128:(tt + 1) * 128, :])
                    fraw.append(t)

                # W1f -> bf16  (rows 0:1024 of W1)
                W1fb = []
                for h in range(HC):
                    w1raw = spool.tile([128, J], f32, tag="w1raw")
                    nc.sync.dma_start(out=w1raw[:], in_=W1_d[h * 128:(h + 1) * 128, :])
                    t = cpool.tile([128, J], bf16, tag=f"w1fb{h}")
                    nc.vector.tensor_copy(t[:], w1raw[:])
                    W1fb.append(t)

                # f -> f^T (PE transpose, fp32 in -> psum -> bf16 sbuf)
                for h in range(HC):
                    ft = cpool.tile([128, T], bf16, tag=f"fT{h}")
                    for tt in range(2):
                        pt = ppA.tile([128, 128], f32, tag="tp")
                        nc.tensor.transpose(pt[:], fraw[tt][:, h * 128:(h + 1) * 128],
                                            identity[:])
                        nc.vector.tensor_copy(ft[:, tt * 128:(tt + 1) * 128], pt[:])
                    fTb.append(ft)

                # hf^T = W1f^T @ f^T
                for c in range(JC):
                    pf = ppC.tile([128, T], f32, tag="pf")
                    for h in range(HC):
                        nc.tensor.matmul(pf[:], W1fb[h][:, c * 128:(c + 1) * 128],
                                         fTb[h][:], start=(h == 0), stop=(h == HC - 1))
                    t = cpool.tile([128, T], f32, tag=f"hfT{c}")
                    nc.vector.tensor_copy(t[:], pf[:])
                    hfTs.append(t)

                # W2 -> bf16
                for c in range(JC):
                    w2raw = spool.tile([128, V], f32, tag="w2raw")
                    nc.sync.dma_start(out=w2raw[:], in_=W2_d[c * 128:(c + 1) * 128, :])
                    t = cpool.tile([128, V], bf16, tag=f"w2b{c}")
                    nc.vector.tensor_copy(t[:], w2raw[:])
                    W2b.append(t)

                # W1g -> bf16  (rows 1024:1344)
                W1gb = []
                for pc, (po, pn) in enumerate(g_chunks):
                    w1graw = spool.tile([pn, J], f32, tag="w1graw")
                    nc.sync.dma_start(out=w1graw[:], in_=W1_d[EH + po:EH + po + pn, :])
                    t = cpool.tile([pn, J], bf16, tag=f"w1gb{pc}")
                    nc.vector.tensor_copy(t[:], w1graw[:])
                    W1gb.append(t)

                # b1 as [128, JC] (partition p, chunk c) ; b2 row
                b1sb = cpool.tile([128, JC], f32)
                nc.sync.dma_start(out=b1sb[:], in_=b1_d[:].rearrange("(c p) -> p c", p=128))
                b2row = cpool.tile([1, V], f32)
                nc.sync.dma_start(out=b2row[:], in_=b2_d[:].rearrange("(a v) -> a v", a=1))

                # broadcast b2 across 128 partitions via rank-1 matmul
                ones = cpool.tile([1, 128], f32)
                nc.vector.memset(ones[:], 1.0)
                b2bc = cpool.tile([128, V], f32)
                for vh in range(2):
                    pb = ppB.tile([128, 512], f32, tag="pb")
                    nc.tensor.matmul(pb[:], ones[:], b2row[:, vh * 512:(vh + 1) * 512],
                                     start=True, stop=True)
                    nc.vector.tensor_copy(b2bc[:, vh * 512:(vh + 1) * 512], pb[:])

                # g -> g^T
                graw = spool.tile([U, PH], f32, tag="graw")
                nc.sync.dma_start(out=graw[:], in_=g_d[:])
                gTb = []
                for pc, (po, pn) in enumerate(g_chunks):
                    pt = ppA.tile([128, U], f32, tag="tp2")
                    nc.tensor.transpose(pt[:pn, :], graw[:, po:po + pn],
                                        identity[:U, :U])
                    t = cpool.tile([pn, U], bf16, tag=f"gT{pc}")
                    nc.vector.tensor_copy(t[:], pt[:pn, :])
                    gTb.append(t)

                # hg^T = W1g^T @ g^T  (+ b1, fused on drain)
                for c in range(JC):
                    ph = ppB.tile([128, U], f32, tag="ph")
                    for pc in range(3):
                        nc.tensor.matmul(ph[:], W1gb[pc][:, c * 128:(c + 1) * 128],
                                         gTb[pc][:], start=(pc == 0), stop=(pc == 2))
                    t = cpool.tile([128, U], f32, tag=f"hgT{c}")
                    nc.vector.tensor_scalar(t[:], ph[:], b1sb[:, c:c + 1], None, add)
                    hgTs.append(t)

            # ---------------- main loop over u ----------------
            with (
                tc.tile_pool(name="hpool", bufs=4) as hpool,
                tc.tile_pool(name="opool", bufs=4) as opool,
                tc.tile_pool(name="mpsum", bufs=2, space=bass.MemorySpace.PSUM) as mpsum,
            ):
                for u in range(U):
                    Hs = []
                    for c in range(JC):
                        ht = hpool.tile([128, T], bf16, tag=f"H{c}")
                        nc.scalar.activation(ht[:], hfTs[c][:], Relu,
                                             bias=hgTs[c][:, u:u + 1], scale=1.0)
                        Hs.append(ht)
                    for tt in range(2):
                        ps0 = mpsum.tile([128, 512], f32, tag=f"ps{tt}0")
                        ps1 = mpsum.tile([128, 512], f32, tag=f"ps{tt}1")
                        ps = [ps0, ps1]
                        for c in range(JC):
                            lhsT = Hs[c][:, tt * 128:(tt + 1) * 128]
                            nc.tensor.matmul(ps[0][:], lhsT, W2b[c][:, 0:512],
                                             start=(c == 0), stop=(c == JC - 1))
                            nc.tensor.matmul(ps[1][:], lhsT, W2b[c][:, 512:1024],
                                             start=(c == 0), stop=(c == JC - 1))
                        for vh in range(2):
                            ot = opool.tile([128, 512], f32, tag=f"o{tt}{vh}")
                            nc.vector.tensor_tensor(
                                ot[:], ps[vh][:],
                                b2bc[:, vh * 512:(vh + 1) * 512], add)
                            nc.sync.dma_start(
                                out=out_d[tt * 128:(tt + 1) * 128, u,
                                          vh * 512:(vh + 1) * 512],
                                in_=ot[:])
    nc.compile()
    return nc


def _get_nc():
    if "nc" not in _CACHE:
        _CACHE["nc"] = _build_nc()
    return _CACHE["nc"]


def run(f, g, W1, b1, W2, b2, trace=False):
    """Returns (full_output, BassKernelResults)."""
    from concourse.bass_utils import run_bass_kernel_spmd

    nc = _get_nc()
    in_maps = []
    for i in range(N_CORES):
        in_maps.append({
            "f": np.ascontiguousarray(f[i], dtype=np.float32),
            "g": np.ascontiguousarray(g[i], dtype=np.float32),
            "W1": np.ascontiguousarray(W1, dtype=np.float32),
            "b1": np.ascontiguousarray(b1, dtype=np.float32),
            "W2": np.ascontiguousarray(W2, dtype=np.float32),
            "b2": np.ascontiguousarray(b2, dtype=np.float32),
        })
    res = run_bass_kernel_spmd(nc, in_maps, list(range(N_CORES)), trace=trace)
    out = np.stack([res.results[i]["out"] for i in range(N_CORES)], axis=0)
    return out, res


def kernel(f, g, W1, b1, W2, b2):
    out, _ = run(f, g, W1, b1, W2, b2)
    return out

